# revision 1
# baseline (speedup 1.0000x reference)
"""QSP expectation kernel for Trainium2 (Bass/Tile), 8-core data parallel.

Math: for the QSP sequence U = S(phi_0) * prod_{k=1..2d} [W(x) S(phi_k)] with
d=10, the output Re(U[0,0]) is exactly a degree-10 trigonometric polynomial in
theta = 2x:

    g(x) = a0 + sum_{m=1..10} A_m * sin(2m*x + ph_m)

The 21 coefficients (a0, A_1..10, ph_1..10) are recovered from the 21 phase
params by sampling the (tiny) recurrence at 64 points in float64 and taking an
FFT — exact to machine precision (residual harmonics vanish identically).

Hardware Sin (ScalarE spline) is only valid for |arg| <= ~pi, so all Sin
arguments are pre-reduced. The host (float64, exact) ships the four head
angles a_m = wrap(m*2x + ph_m), m=1..4, plus the tail step d4 = wrap(8x) and
alphas — DMA has headroom, VectorE does not. The device derives the six tail
angles with four parallel chains a_m = wrap(a_{m-4} + d4 + dph), each add
bounded by 3pi so the ADD_RANGE_WRAP custom DVE op (shift, then wrap by one
2pi into [-pi, pi]) suffices. Sin terms are accumulated in two half-chains of
fused scalar_tensor_tensor ops, combined, and scaled by alphas. All
elementwise work is VectorE/ScalarE; walrus rejects TensorTensor-class
opcodes on GpSimd in this toolchain, and VectorE is the saturated engine.
"""

import numpy as np

N = 4_000_000
NCORES = 8
PER = N // NCORES          # 500_000 elements per core
P = 128                    # SBUF partitions
FD = 3912                  # free dim per core; PER=500000 padded to P*FD=500736
NT = 4                     # column tiles
TFD = FD // NT             # 978
DEPTH = 10
NH = 10                    # harmonics 1..10

PI = float(np.float32(np.pi))
TWO_PI = float(np.float32(2 * np.pi))

_cache = {}


def _trig_coeffs(phi):
    """Exact harmonic decomposition of the QSP expectation, in float64."""
    phi = np.asarray(phi, dtype=np.float64)
    nfft = 64
    theta = 2 * np.pi * np.arange(nfft) / nfft
    x = theta / 2
    c = np.cos(x)
    s = np.sin(x)
    a = np.exp(1j * phi[0]) * np.ones_like(x, dtype=np.complex128)
    b = np.zeros_like(a)
    for k in range(1, 2 * DEPTH + 1):
        p = np.exp(1j * phi[k])
        ta = a * c + b * (1j * s)
        tb = a * (1j * s) + b * c
        a = ta * p
        b = tb * np.conj(p)
    g = a.real  # Re(U[0,0]) on the sample grid
    F = np.fft.rfft(g) / nfft
    a0 = F[0].real
    am = 2 * F.real          # cos(m theta) coefficients
    bm = -2 * F.imag         # sin(m theta) coefficients
    A = np.hypot(am, bm)[1 : NH + 1]
    ph = np.arctan2(am, bm)[1 : NH + 1]
    return float(a0), A, ph


def _wrap_pi(v):
    """Centered mod into [-pi, pi)."""
    return np.mod(np.asarray(v, np.float64) + np.pi, 2 * np.pi) - np.pi


def _build_nc(a0, A, ph, nt=NT, gp_add=False, gp_acc=0, gp_mul=False):
    """gp_add: angle-chain tensor_adds on GpSimd; gp_acc: how many of the 9
    accumulation STTs go to GpSimd; gp_mul: final alpha-multiply on GpSimd."""
    import concourse.bacc as bacc
    import concourse.mybir as mybir
    import concourse.tile as tile

    f32 = mybir.dt.float32
    Sin = mybir.ActivationFunctionType.Sin
    mult = mybir.AluOpType.mult
    add = mybir.AluOpType.add

    tfd = FD // nt

    # Per-step phase increments, pre-wrapped so |a_prev + d + dph| <= 3pi.
    dph = _wrap_pi(np.diff(ph))

    nc = bacc.Bacc()
    ains = [
        nc.dram_tensor(f"a{i}", [P, FD], f32, kind="ExternalInput")
        for i in range(1, 5)
    ]
    x4in = nc.dram_tensor("x4", [P, FD], f32, kind="ExternalInput")
    alf = nc.dram_tensor("alphas", [P, FD], f32, kind="ExternalInput")
    out = nc.dram_tensor("out", [P, FD], f32, kind="ExternalOutput")

    with tile.TileContext(nc) as tc:
        with (
            tc.tile_pool(name="io", bufs=3) as io_pool,
            tc.tile_pool(name="ain", bufs=2) as ain_pool,
            tc.tile_pool(name="ang", bufs=8) as ang_pool,
            tc.tile_pool(name="raw", bufs=4) as raw_pool,
            tc.tile_pool(name="terms", bufs=6) as term_pool,
            tc.tile_pool(name="acc", bufs=6) as acc_pool,
            tc.tile_pool(name="tot", bufs=2) as tot_pool,
        ):
            for t in range(nt):
                sl = slice(t * tfd, (t + 1) * tfd)
                at = io_pool.tile([P, tfd], f32, tag="at")
                nc.sync.dma_start(out=at[:], in_=alf[:, sl])
                d4 = io_pool.tile([P, tfd], f32, tag="d4")
                nc.sync.dma_start(out=d4[:], in_=x4in[:, sl])

                add_eng = nc.gpsimd if gp_add else nc.vector

                def wrapped(src, shift, tag="ang"):
                    o = ang_pool.tile([P, tfd], f32, tag=tag)
                    nc.vector.add_range_wrap(o[:], src[:], float(shift), PI, TWO_PI)
                    return o

                def add_wrap(x1, x2, shift):
                    raw = raw_pool.tile([P, tfd], f32, tag="raw")
                    add_eng.tensor_add(raw[:], x1[:], x2[:])
                    return wrapped(raw, shift)

                # Head angles a1..a4 = wrap(m*theta + ph_m) come from the
                # host; four parallel tail chains step by d4 = wrap(8x).
                a = [None] * (NH + 1)
                for i in range(1, 5):
                    head = ain_pool.tile([P, tfd], f32, tag=f"ain{i}")
                    nc.sync.dma_start(out=head[:], in_=ains[i - 1][:, sl])
                    a[i] = head
                for m in range(5, NH + 1):
                    a[m] = add_wrap(a[m - 4], d4, _wrap_pi(ph[m - 1] - ph[m - 5]))

                terms = [None] * (NH + 1)
                for m in range(1, NH + 1):
                    term = term_pool.tile([P, tfd], f32, tag="term")
                    nc.scalar.activation(term[:], a[m][:], Sin, bias=0.0, scale=1.0)
                    terms[m] = term

                # Two accumulation half-chains, combined at the end.
                def half_acc(ms, base, n_gp):
                    acc = None
                    for i, m in enumerate(ms):
                        nacc = acc_pool.tile([P, tfd], f32, tag="acc")
                        if acc is None:
                            nc.vector.tensor_scalar(
                                nacc[:], terms[m][:], float(A[m - 1]), float(base),
                                mult, add,
                            )
                        else:
                            eng = nc.gpsimd if i <= n_gp else nc.vector
                            eng.scalar_tensor_tensor(
                                nacc[:], terms[m][:], float(A[m - 1]), acc[:],
                                mult, add,
                            )
                        acc = nacc
                    return acc

                acc_a = half_acc([1, 3, 5, 7, 9], a0, gp_acc)
                acc_b = half_acc([2, 4, 6, 8, 10], 0.0, gp_acc)
                tot = tot_pool.tile([P, tfd], f32, tag="tot")
                (nc.gpsimd if gp_mul else nc.vector).tensor_add(
                    tot[:], acc_a[:], acc_b[:]
                )
                ot = io_pool.tile([P, tfd], f32, tag="ot")
                (nc.gpsimd if gp_mul else nc.vector).tensor_mul(ot[:], tot[:], at[:])
                nc.sync.dma_start(out=out[:, sl], in_=ot[:])
    nc.finalize()
    return nc


def _get_runner(key):
    if key not in _cache:
        phi = np.frombuffer(key, dtype=np.float32)
        a0, A, ph = _trig_coeffs(phi)
        _cache[key] = _build_nc(a0, A, ph)
    return _cache[key]


def kernel(x, qsp_params, alphas):
    from concourse.bass_utils import run_bass_kernel_spmd

    x = np.asarray(x, dtype=np.float32).reshape(-1)
    alphas = np.ascontiguousarray(np.asarray(alphas, dtype=np.float32).reshape(-1))
    qsp_params = np.asarray(qsp_params, dtype=np.float32).reshape(-1)
    assert x.shape[0] == N and alphas.shape[0] == N

    nc = _get_runner(qsp_params.tobytes())

    # Host-side range reductions: head angles wrap(m*2x + ph_m) for m=1..4
    # and the tail step d4 = centered_mod(8x, 2pi).
    phi = qsp_params
    a0_, A_, ph_ = _trig_coeffs(phi)
    xf = x.astype(np.float64)
    theta = 2.0 * xf
    heads = [_wrap_pi(m * theta + ph_[m - 1]).astype(np.float32) for m in range(1, 5)]
    d4 = _wrap_pi(4.0 * theta).astype(np.float32)

    pad = P * FD - PER
    in_maps = []
    for c in range(NCORES):
        cs = slice(c * PER, (c + 1) * PER)
        m_ = {
            f"a{i}": np.pad(heads[i - 1][cs], (0, pad)).reshape(P, FD)
            for i in range(1, 5)
        }
        m_["x4"] = np.pad(d4[cs], (0, pad)).reshape(P, FD)
        m_["alphas"] = np.pad(alphas[cs], (0, pad)).reshape(P, FD)
        in_maps.append(m_)

    res = run_bass_kernel_spmd(nc, in_maps, core_ids=list(range(NCORES)))
    outs = [r["out"].reshape(-1)[:PER] for r in res.results]
    return np.concatenate(outs).astype(np.float32)[:, None]



# revision 2
# speedup vs baseline: 4.0419x; 4.0419x over previous
"""QSP expectation kernel for Trainium2 (Bass/Tile), 8-core data parallel.

Math: Re(U[0,0]) of the QSP sequence is an EVEN trig polynomial of theta=2x:
g(x) = a0 + sum_m gamma_m cos(2m x)  (the sin components vanish structurally:
U00 = P(cos x) with real part even in x). With c = cos(2x) this is a single
degree-K polynomial p in c (Chebyshev recombination), and with the host
shipping the range-reduced u = x - pi*round(x/pi) in fp16 (same wrap-only
precompute the previous version used for its head angles), the device needs
only s = Sin(u) (ScalarE, |u| <= pi/2 well inside the table range) and
y = s^2, since c = 1 - 2y.

p is factored over its roots into real quadratics (y+b)^2 + d (complex pairs
directly; real roots paired up; odd-degree leftovers become linear factors).
Each quadratic's square lands on ScalarE (Square activation with bias), and
each "+d then multiply into the running product" is one fused VectorE
scalar_tensor_tensor. The leading coefficient rides on alphas from the host
(al~ = q_K * alpha), so the whole per-element pipeline is:

  ACT:  s = Sin(u); g_i = Square(y + b_i)          (K/2+1 ops)
  DVE:  y = s*s;    acc = (g_i + d_i) * acc        (K/2+1 ops)

All tiles fp16 (DVE tensor_tensor runs 2x on 2-byte dtypes; DMA bytes halve);
the ALU datapaths are fp32 internally so rounding only hits tile stores.
Truncation K is chosen adaptively from the harmonic amplitudes (rel L2
truncation err < 6e-3 of the signal rms; tolerance is 2e-2).
"""

import numpy as np

N = 4_000_000
NCORES = 8
PER = N // NCORES          # 500_000 elements per core
P = 128                    # SBUF partitions
FD = 3912                  # free dim per core; PER=500000 padded to P*FD=500736
NT = 4                     # column tiles
DEPTH = 10
NH = 2 * DEPTH + 1

PI64 = np.float64(np.pi)

_cache = {}


def _cos_series(phi):
    """Exact cos-series of the QSP expectation: g(theta) = a0 + sum gam_m
    cos(m theta), computed in float64 via the 2x2 recurrence on a 64-pt grid
    + rFFT. Asserts the sin components vanish (structural: g is even)."""
    phi = np.asarray(phi, dtype=np.float64)
    nfft = 64
    theta = 2 * np.pi * np.arange(nfft) / nfft
    x = theta / 2
    c = np.cos(x)
    s = np.sin(x)
    a = np.exp(1j * phi[0]) * np.ones_like(x, dtype=np.complex128)
    b = np.zeros_like(a)
    for k in range(1, NH):
        p = np.exp(1j * phi[k])
        ta = a * c + b * (1j * s)
        tb = a * (1j * s) + b * c
        a = ta * p
        b = tb * np.conj(p)
    g = a.real
    F = np.fft.rfft(g) / nfft
    a0 = F[0].real
    gam = 2 * F.real[1 : DEPTH + 1]       # cos(m theta) coefficients
    sin_part = -2 * F.imag[1 : DEPTH + 1]
    assert np.abs(sin_part).max() < 1e-9, "sin components should vanish"
    return float(a0), gam


def _poly_y(a0, gam, K):
    """p(y) coefficients (ascending, float64) with y = sin^2(x mod pi),
    i.e. p(y) = P(1-2y) where P(c) = a0 + sum_{m<=K} gam_m T_m(c)."""
    from numpy.polynomial import chebyshev as C, polynomial as Po

    cheb = np.zeros(K + 1)
    cheb[0] = a0
    cheb[1 : K + 1] = gam[:K]
    pc = C.cheb2poly(cheb)
    py = np.zeros(1)
    for i, co in enumerate(pc):
        py = Po.polyadd(py, co * Po.polypow([1.0, -2.0], i))
    return py


def _pick_K(a0, gam):
    rms = np.sqrt(a0**2 + (gam**2).sum() / 2)
    for K in range(4, DEPTH + 1):
        tail = np.sqrt((gam[K:] ** 2).sum() / 2)
        if tail < 6e-3 * rms:
            return K
    return DEPTH


def _factorize(py):
    """Factor p(y) = lead * prod[(y+b)^2 + d] * prod[(y - r)] over real
    quadratics. Complex root pairs map directly; real roots are paired in
    sorted order; an odd leftover real root becomes a linear factor."""
    lead = py[-1]
    roots = np.roots(py[::-1])
    quads = []
    reals = []
    used = np.zeros(len(roots), bool)
    for i, r in enumerate(roots):
        if used[i]:
            continue
        if abs(r.imag) > 1e-9:
            # find conjugate partner
            j = np.argmin(np.abs(roots - np.conj(r)) + used * 1e9)
            used[i] = used[j] = True
            quads.append((float(-r.real), float(r.imag**2)))
        else:
            used[i] = True
            reals.append(float(r.real))
    reals.sort()
    while len(reals) >= 2:
        r1 = reals.pop()
        r2 = reals.pop()
        quads.append((float(-(r1 + r2) / 2), float(-((r1 - r2) / 2) ** 2)))
    return float(lead), quads, reals


def _build_nc(quads, lins, nt=NT):
    import concourse.bacc as bacc
    import concourse.mybir as mybir
    import concourse.tile as tile

    f16 = mybir.dt.float16
    f32 = mybir.dt.float32
    Sin = mybir.ActivationFunctionType.Sin
    Square = mybir.ActivationFunctionType.Square
    mult = mybir.AluOpType.mult
    add = mybir.AluOpType.add

    tfd = FD // nt

    nc = bacc.Bacc()
    uin = nc.dram_tensor("u", [P, FD], f16, kind="ExternalInput")
    alf = nc.dram_tensor("al", [P, FD], f16, kind="ExternalInput")
    out = nc.dram_tensor("out", [P, FD], f16, kind="ExternalOutput")

    # fp32 const APs for the Square biases (only 0.0/1.0 are pre-registered).
    bias_aps = {}
    for b, _ in quads:
        if (mybir.dt.float32, float(b)) not in nc.const_aps.aps:
            t = nc.alloc_sbuf_tensor(f"const-b-{len(bias_aps)}", [P, 1], f32)
            nc.gpsimd.memset(t.ap(), float(b))
            nc.const_aps.aps[(mybir.dt.float32, float(b))] = t.ap()
            bias_aps[float(b)] = t.ap()
    nc.all_engine_barrier()

    with tile.TileContext(nc) as tc:
        with (
            tc.tile_pool(name="io", bufs=3) as io_pool,
            tc.tile_pool(name="trig", bufs=3) as trig_pool,
            tc.tile_pool(name="sq", bufs=4) as sq_pool,
            tc.tile_pool(name="acc", bufs=4) as acc_pool,
        ):
            for t in range(nt):
                sl = slice(t * tfd, (t + 1) * tfd)
                ut = io_pool.tile([P, tfd], f16, tag="ut")
                nc.sync.dma_start(out=ut[:], in_=uin[:, sl])
                at = io_pool.tile([P, tfd], f16, tag="at")
                nc.sync.dma_start(out=at[:], in_=alf[:, sl])

                s = trig_pool.tile([P, tfd], f16, tag="s")
                nc.scalar.activation(s[:], ut[:], Sin, bias=0.0, scale=1.0)
                y = trig_pool.tile([P, tfd], f16, tag="y")
                nc.vector.tensor_mul(y[:], s[:], s[:])

                acc = at
                for b, d in quads:
                    g = sq_pool.tile([P, tfd], f16, tag="g")
                    nc.scalar.activation(g[:], y[:], Square, bias=float(b),
                                         scale=1.0)
                    nacc = acc_pool.tile([P, tfd], f16, tag="acc")
                    nc.vector.scalar_tensor_tensor(
                        nacc[:], g[:], float(d), acc[:], add, mult
                    )
                    acc = nacc
                for r in lins:
                    nacc = acc_pool.tile([P, tfd], f16, tag="acc")
                    nc.vector.scalar_tensor_tensor(
                        nacc[:], y[:], float(-r), acc[:], add, mult
                    )
                    acc = nacc
                nc.sync.dma_start(out=out[:, sl], in_=acc[:])
    nc.finalize()
    return nc


def _prep(key):
    if key not in _cache:
        phi = np.frombuffer(key, dtype=np.float32)
        a0, gam = _cos_series(phi)
        K = _pick_K(a0, gam)
        py = _poly_y(a0, gam, K)
        lead, quads, lins = _factorize(py)
        nc = _build_nc(quads, lins)
        _cache[key] = (nc, lead)
    return _cache[key]


def _get_runner(key):
    return _prep(key)[0]


def kernel(x, qsp_params, alphas):
    from concourse.bass_utils import run_bass_kernel_spmd

    x = np.asarray(x, dtype=np.float32).reshape(-1)
    alphas = np.asarray(alphas, dtype=np.float32).reshape(-1)
    qsp_params = np.asarray(qsp_params, dtype=np.float32).reshape(-1)
    assert x.shape[0] == N and alphas.shape[0] == N

    nc, lead = _prep(qsp_params.tobytes())

    # Host range reduction (wrap only): u = x - pi*round(x/pi) in [-pi/2,pi/2];
    # cos(2x) = cos(2u) = 1 - 2 sin^2(u). Leading coeff rides on alphas.
    xf = x.astype(np.float64)
    u = (xf - PI64 * np.round(xf / PI64)).astype(np.float16)
    al = (alphas.astype(np.float64) * lead).astype(np.float16)

    pad = P * FD - PER
    in_maps = []
    for c in range(NCORES):
        cs = slice(c * PER, (c + 1) * PER)
        in_maps.append({
            "u": np.pad(u[cs], (0, pad)).reshape(P, FD),
            "al": np.pad(al[cs], (0, pad)).reshape(P, FD),
        })

    res = run_bass_kernel_spmd(nc, in_maps, core_ids=list(range(NCORES)))
    outs = [r["out"].reshape(-1)[:PER] for r in res.results]
    return np.concatenate(outs).astype(np.float32)[:, None]


# revision 5
# speedup vs baseline: 5.6423x; 1.3960x over previous
"""QSP expectation kernel for Trainium2 (Bass/Tile), 8-core data parallel.

Math: Re(U[0,0]) of the QSP sequence is an EVEN trig polynomial of theta=2x
(structural: U00 = P(cos x) has even real part), so with c = cos(2x) the
output is a single degree-K polynomial in c. The host ships only the
range-reduced u = x - pi*round(x/pi) (fp16) and alphas (fp16, pre-scaled by
the leading coefficient); the device computes s = Sin(u) (ScalarE) and
evaluates the polynomial over y = s^2 (c = 1 - 2y) factored into real
quadratics (y+b)^2 + d.

The quadratic factors are fused into two custom DVE ops (registered at
import via the documented dve_ops authoring API; the uop program is written
into the per-NEFF DVE table, no firmware change):

  QSP_QUAD_MUL:  out = ((s^2 + b)^2 + d) * acc        (5 ALU stages)
  QSP_PAIR:      out = ((s^2+b2)^2+d2)*((s^2+b3)^2+d3) (8 stages, d3 via the
                 C3->Src1 latch spill, a [P,1] const AP)

so a degree-6 evaluation is 3 VectorE ops per tile (QUAD_MUL, PAIR, one fp16
tensor_tensor for the final product) plus a single ScalarE Sin. Truncation K
is chosen adaptively (rel L2 truncation err < 6e-3 of signal rms; tolerance
2e-2); non-degree-6 parameter sets fall back to a native Square/STT chain.
"""

import numpy as np

N = 4_000_000
NCORES = 8
PER = N // NCORES          # 500_000 elements per core
P = 128                    # SBUF partitions
FD = 3912                  # free dim per core; PER=500000 padded to P*FD=500736
DEPTH = 10
NH = 2 * DEPTH + 1

# Non-uniform column tiles: small first tile so compute starts early, small
# last tile so the final store+sem tail is short.
TILES = [489, 978, 978, 978, 489]
assert sum(TILES) == FD

PI64 = np.float64(np.pi)

_cache = {}
_ops = {}


def _register_custom_ops():
    """Register the two fused quadratic-factor ops in concourse.dve_ops via
    the documented authoring pattern (OPS.append + sub-opcode row); shas are
    computed from the actual lowering so compile()'s pin check passes."""
    if _ops:
        return _ops
    import concourse.dve_ops as dve_ops
    from concourse.dve_ops import DveOp
    from concourse.dve_spec import (
        C0, C1, C2, C3, Spec, Src0, Src1, _spill_c3_to_src1, lower, sq,
    )
    from concourse.dve_spec import _has_src1
    from concourse.dve_uop import DveOpSpec

    if "QSP_QUAD_MUL" in dve_ops._SUB_OPCODE_FOR_NAME:
        _ops["qm"] = next(o for o in dve_ops.OPS if o.name == "QSP_QUAD_MUL")
        _ops["pair"] = next(o for o in dve_ops.OPS if o.name == "QSP_PAIR")
        return _ops

    used_rows = set(dve_ops._SUB_OPCODE_FOR_NAME.values())
    try:
        from concourse.dve_table_gen import free_opcode_rows
        free = [r for r in free_opcode_rows("TRN2") if r not in used_rows]
    except Exception:
        free = [r for r in range(1, 0x20) if r not in used_rows]
    rows = free[:2]
    assert len(rows) == 2, "no free DVE opcode rows"

    def ref_qm(in0, in1, s0, s1, imm2):
        y = in0.astype(np.float32) * in0.astype(np.float32)
        return (((y + s0) * (y + s0) + s1) * in1).astype(np.float32)

    def ref_pair(in0, in1, s0, s1, imm2):
        y = in0.astype(np.float32) * in0.astype(np.float32)
        f2 = (y + s0) * (y + s0) + s1
        f3 = (y + imm2) * (y + imm2) + np.asarray(in1, np.float32).reshape(-1, 1)
        return (f2 * f3).astype(np.float32)

    y = Src0 * Src0
    spec_qm = Spec(body=(sq(y + C0) + C1) * Src1, reference=ref_qm)
    spec_pair = Spec(
        body=_spill_c3_to_src1((sq(y + C0) + C1) * (sq(y + C2) + C3)),
        reference=ref_pair,
    )

    made = []
    for name, row, spec in (
        ("QSP_QUAD_MUL", rows[0], spec_qm),
        ("QSP_PAIR", rows[1], spec_pair),
    ):
        dve_ops._SUB_OPCODE_FOR_NAME[name] = row
        shas = {}
        for ver in ("v3", "v4"):
            try:
                d = DveOpSpec(name=name, opcode=row, uops=lower(spec, ver=ver),
                              rd1_en=_has_src1(spec))
                shas[ver] = d.sha(ver)
            except Exception:
                pass
        assert "v3" in shas, f"{name}: v3 lowering failed"
        op = DveOp(name, spec, subdim=False, uops_sha=shas)
        dve_ops.OPS.append(op)
        dve_ops.CUSTOM_DVE_SPECS[name] = spec
        made.append(op)
    _ops["qm"], _ops["pair"] = made
    return _ops


def _cos_series(phi):
    """Exact cos-series of the QSP expectation in float64: g = a0 + sum
    gam_m cos(m theta) via the 2x2 recurrence on a 64-pt grid + rFFT."""
    phi = np.asarray(phi, dtype=np.float64)
    nfft = 64
    theta = 2 * np.pi * np.arange(nfft) / nfft
    x = theta / 2
    c = np.cos(x)
    s = np.sin(x)
    a = np.exp(1j * phi[0]) * np.ones_like(x, dtype=np.complex128)
    b = np.zeros_like(a)
    for k in range(1, NH):
        p = np.exp(1j * phi[k])
        ta = a * c + b * (1j * s)
        tb = a * (1j * s) + b * c
        a = ta * p
        b = tb * np.conj(p)
    g = a.real
    F = np.fft.rfft(g) / nfft
    a0 = F[0].real
    gam = 2 * F.real[1 : DEPTH + 1]
    sin_part = -2 * F.imag[1 : DEPTH + 1]
    assert np.abs(sin_part).max() < 1e-9, "sin components should vanish"
    return float(a0), gam


def _poly_y(a0, gam, K):
    """p(y) coefficients (ascending, float64): p(y) = P(1-2y),
    P(c) = a0 + sum_{m<=K} gam_m T_m(c)."""
    from numpy.polynomial import chebyshev as C, polynomial as Po

    cheb = np.zeros(K + 1)
    cheb[0] = a0
    cheb[1 : K + 1] = gam[:K]
    pc = C.cheb2poly(cheb)
    py = np.zeros(1)
    for i, co in enumerate(pc):
        py = Po.polyadd(py, co * Po.polypow([1.0, -2.0], i))
    return py


def _pick_K(a0, gam):
    rms = np.sqrt(a0**2 + (gam**2).sum() / 2)
    for K in range(4, DEPTH + 1):
        tail = np.sqrt((gam[K:] ** 2).sum() / 2)
        if tail < 6e-3 * rms:
            return K
    return DEPTH


def _factorize(py):
    """p(y) = lead * prod[(y+b)^2 + d] * prod[(y - r)]: complex root pairs
    map directly; real roots pair up in sorted order; an odd leftover
    becomes a linear factor."""
    lead = py[-1]
    roots = np.roots(py[::-1])
    quads = []
    reals = []
    used = np.zeros(len(roots), bool)
    for i, r in enumerate(roots):
        if used[i]:
            continue
        if abs(r.imag) > 1e-9:
            j = np.argmin(np.abs(roots - np.conj(r)) + used * 1e9)
            used[i] = used[j] = True
            quads.append((float(-r.real), float(r.imag**2)))
        else:
            used[i] = True
            reals.append(float(r.real))
    reals.sort()
    while len(reals) >= 2:
        r1 = reals.pop()
        r2 = reals.pop()
        quads.append((float(-(r1 + r2) / 2), float(-((r1 - r2) / 2) ** 2)))
    return float(lead), quads, reals


def _build_nc(quads, lins):
    import concourse.bacc as bacc
    import concourse.mybir as mybir
    import concourse.tile as tile

    ops = _register_custom_ops()

    f16 = mybir.dt.float16
    f32 = mybir.dt.float32
    Sin = mybir.ActivationFunctionType.Sin
    Square = mybir.ActivationFunctionType.Square
    mult = mybir.AluOpType.mult
    add = mybir.AluOpType.add

    fast = len(quads) == 3 and len(lins) == 0

    nc = bacc.Bacc()
    uin = nc.dram_tensor("u", [P, FD], f16, kind="ExternalInput")
    alf = nc.dram_tensor("al", [P, FD], f16, kind="ExternalInput")
    out = nc.dram_tensor("out", [P, FD], f16, kind="ExternalOutput")

    d3_ap = None
    if fast:
        t = nc.alloc_sbuf_tensor("const-d3", [P, 1], f32)
        nc.gpsimd.memset(t.ap(), float(quads[2][1]))
        d3_ap = t.ap()
    else:
        for b, _ in quads:
            if (f32, float(b)) not in nc.const_aps.aps:
                t = nc.alloc_sbuf_tensor(f"const-b-{b}", [P, 1], f32)
                nc.gpsimd.memset(t.ap(), float(b))
                nc.const_aps.aps[(f32, float(b))] = t.ap()
    nc.all_engine_barrier()

    nt = len(TILES)
    offs = np.concatenate([[0], np.cumsum(TILES)]).astype(int)

    with tile.TileContext(nc) as tc:
        with (
            tc.tile_pool(name="io", bufs=2 * nt) as io_pool,
            tc.tile_pool(name="trig", bufs=3) as trig_pool,
            tc.tile_pool(name="sq", bufs=4) as sq_pool,
            tc.tile_pool(name="acc", bufs=4) as acc_pool,
        ):
            uts, ats = [], []
            for t in range(nt):
                sl = slice(offs[t], offs[t + 1])
                ut = io_pool.tile([P, TILES[t]], f16, tag=f"ut{t}")
                nc.sync.dma_start(out=ut[:], in_=uin[:, sl])
                uts.append(ut)
            for t in range(nt):
                sl = slice(offs[t], offs[t + 1])
                at = io_pool.tile([P, TILES[t]], f16, tag=f"at{t}")
                nc.sync.dma_start(out=at[:], in_=alf[:, sl])
                ats.append(at)
            for t in range(nt):
                tfd = TILES[t]
                sl = slice(offs[t], offs[t + 1])
                s = trig_pool.tile([P, tfd], f16, tag="s")
                nc.scalar.activation(s[:], uts[t][:], Sin, bias=0.0, scale=1.0)

                if fast:
                    (b1, d1), (b2, d2), (b3, _) = quads
                    p23 = sq_pool.tile([P, tfd], f16, tag="p23")
                    nc.vector._custom_dve(
                        ops["pair"], out=p23[:], in0=s[:], in1=d3_ap,
                        s0=float(b2), s1=float(d2), imm2=float(b3),
                    )
                    q1 = acc_pool.tile([P, tfd], f16, tag="q1")
                    nc.vector._custom_dve(
                        ops["qm"], out=q1[:], in0=s[:], in1=ats[t][:],
                        s0=float(b1), s1=float(d1),
                    )
                    ot = acc_pool.tile([P, tfd], f16, tag="ot")
                    nc.vector.tensor_mul(ot[:], q1[:], p23[:])
                    nc.sync.dma_start(out=out[:, sl], in_=ot[:])
                else:
                    y = trig_pool.tile([P, tfd], f16, tag="y")
                    nc.vector.tensor_mul(y[:], s[:], s[:])
                    acc = ats[t]
                    for b, d in quads:
                        g = sq_pool.tile([P, tfd], f16, tag="g")
                        nc.scalar.activation(g[:], y[:], Square, bias=float(b),
                                             scale=1.0)
                        nacc = acc_pool.tile([P, tfd], f16, tag="acc")
                        nc.vector.scalar_tensor_tensor(
                            nacc[:], g[:], float(d), acc[:], add, mult
                        )
                        acc = nacc
                    for r in lins:
                        nacc = acc_pool.tile([P, tfd], f16, tag="acc")
                        nc.vector.scalar_tensor_tensor(
                            nacc[:], y[:], float(-r), acc[:], add, mult
                        )
                        acc = nacc
                    nc.sync.dma_start(out=out[:, sl], in_=acc[:])
    nc.finalize()
    return nc


def _prep(key):
    if key not in _cache:
        phi = np.frombuffer(key, dtype=np.float32)
        a0, gam = _cos_series(phi)
        K = _pick_K(a0, gam)
        py = _poly_y(a0, gam, K)
        lead, quads, lins = _factorize(py)
        nc = _build_nc(quads, lins)
        _cache[key] = (nc, lead)
    return _cache[key]


def _get_runner(key):
    return _prep(key)[0]


def kernel(x, qsp_params, alphas):
    from concourse.bass_utils import run_bass_kernel_spmd

    x = np.asarray(x, dtype=np.float32).reshape(-1)
    alphas = np.asarray(alphas, dtype=np.float32).reshape(-1)
    qsp_params = np.asarray(qsp_params, dtype=np.float32).reshape(-1)
    assert x.shape[0] == N and alphas.shape[0] == N

    nc, lead = _prep(qsp_params.tobytes())

    # Host range reduction (wrap only): u = x - pi*round(x/pi) in [-pi/2,pi/2];
    # cos(2x) = cos(2u) = 1 - 2 sin^2(u). Leading coeff rides on alphas.
    xf = x.astype(np.float64)
    u = (xf - PI64 * np.round(xf / PI64)).astype(np.float16)
    al = (alphas.astype(np.float64) * lead).astype(np.float16)

    pad = P * FD - PER
    in_maps = []
    for c in range(NCORES):
        cs = slice(c * PER, (c + 1) * PER)
        in_maps.append({
            "u": np.pad(u[cs], (0, pad)).reshape(P, FD),
            "al": np.pad(al[cs], (0, pad)).reshape(P, FD),
        })

    res = run_bass_kernel_spmd(nc, in_maps, core_ids=list(range(NCORES)))
    outs = [r["out"].reshape(-1)[:PER] for r in res.results]
    return np.concatenate(outs).astype(np.float32)[:, None]


# revision 27
# speedup vs baseline: 6.5605x; 1.1627x over previous
"""QSP expectation kernel for Trainium2 (Bass/Tile), 8-core data parallel.

Math: Re(U[0,0]) of the QSP sequence is an EVEN trig polynomial of theta=2x
(structural: U00 = P(cos x) has even real part), so with c = cos(2x) the
output is a single degree-K polynomial in c. The host ships only the
range-reduced u = x - pi*round(x/pi) (fp16) and alphas (fp16, pre-scaled by
the leading coefficient); the device computes s = Sin(u) (ScalarE) and
evaluates the polynomial over y = s^2 (c = 1 - 2y) factored into real
quadratics (y+b)^2 + d.

The quadratic factors are fused into two custom DVE ops (registered at
import via the documented dve_ops authoring API; the uop program is written
into the per-NEFF DVE table, no firmware change):

  QSP_QUAD_MUL:  out = ((s^2 + b)^2 + d) * acc        (5 ALU stages)
  QSP_PAIR:      out = ((s^2+b2)^2+d2)*((s^2+b3)^2+d3) (8 stages, d3 via the
                 C3->Src1 latch spill, a [P,1] const AP)

so a degree-6 evaluation is 3 VectorE ops per tile (QUAD_MUL, PAIR, one fp16
tensor_tensor for the final product) plus a single ScalarE Sin. Truncation K
is chosen adaptively (rel L2 truncation err < 6e-3 of signal rms; tolerance
2e-2); non-degree-6 parameter sets fall back to a native Square/STT chain.
"""

import numpy as np

N = 4_000_000
NCORES = 8
PER = N // NCORES          # 500_000 elements per core
P = 128                    # SBUF partitions
FD = 3912                  # free dim per core; PER=500000 padded to P*FD=500736
DEPTH = 10
NH = 2 * DEPTH + 1

# Non-uniform column tiles: small first tile so compute starts early, small
# last tile so the final store+sem tail is short. TT_POOL marks tiles whose
# final product runs on the otherwise-idle GpSimd engine (early tiles, so
# its slower ops drain before the pipeline tail).
TILES = [489, 815, 1141, 1141, 326]
TT_POOL = (0, 1, 2)
assert sum(TILES) == FD

PI64 = np.float64(np.pi)

_cache = {}
_ops = {}


def _register_custom_ops():
    """Register the two fused quadratic-factor ops in concourse.dve_ops via
    the documented authoring pattern (OPS.append + sub-opcode row); shas are
    computed from the actual lowering so compile()'s pin check passes."""
    if _ops:
        return _ops
    import concourse.dve_ops as dve_ops
    from concourse.dve_ops import DveOp
    from concourse.dve_spec import (
        C0, C1, C2, C3, Spec, Src0, Src1, _spill_c3_to_src1, lower, sq,
    )
    from concourse.dve_spec import _has_src1
    from concourse.dve_uop import DveOpSpec

    if "QSP_QUAD_MUL" in dve_ops._SUB_OPCODE_FOR_NAME:
        _ops["qm"] = next(o for o in dve_ops.OPS if o.name == "QSP_QUAD_MUL")
        _ops["pair"] = next(o for o in dve_ops.OPS if o.name == "QSP_PAIR")
        return _ops

    used_rows = set(dve_ops._SUB_OPCODE_FOR_NAME.values())
    try:
        from concourse.dve_table_gen import free_opcode_rows
        free = [r for r in free_opcode_rows("TRN2") if r not in used_rows]
    except Exception:
        free = [r for r in range(1, 0x20) if r not in used_rows]
    rows = free[:2]
    assert len(rows) == 2, "no free DVE opcode rows"

    def ref_qm(in0, in1, s0, s1, imm2):
        y = in0.astype(np.float32) * in0.astype(np.float32)
        return (((y + s0) * (y + s0) + s1) * in1).astype(np.float32)

    def ref_pair(in0, in1, s0, s1, imm2):
        y = in0.astype(np.float32) * in0.astype(np.float32)
        f2 = (y + s0) * (y + s0) + s1
        f3 = (y + imm2) * (y + imm2) + np.asarray(in1, np.float32).reshape(-1, 1)
        return (f2 * f3).astype(np.float32)

    y = Src0 * Src0
    spec_qm = Spec(body=(sq(y + C0) + C1) * Src1, reference=ref_qm)
    spec_pair = Spec(
        body=_spill_c3_to_src1((sq(y + C0) + C1) * (sq(y + C2) + C3)),
        reference=ref_pair,
    )

    made = []
    for name, row, spec in (
        ("QSP_QUAD_MUL", rows[0], spec_qm),
        ("QSP_PAIR", rows[1], spec_pair),
    ):
        dve_ops._SUB_OPCODE_FOR_NAME[name] = row
        shas = {}
        for ver in ("v3", "v4"):
            try:
                d = DveOpSpec(name=name, opcode=row, uops=lower(spec, ver=ver),
                              rd1_en=_has_src1(spec))
                shas[ver] = d.sha(ver)
            except Exception:
                pass
        assert "v3" in shas, f"{name}: v3 lowering failed"
        op = DveOp(name, spec, subdim=False, uops_sha=shas)
        dve_ops.OPS.append(op)
        dve_ops.CUSTOM_DVE_SPECS[name] = spec
        made.append(op)
    _ops["qm"], _ops["pair"] = made
    return _ops


def _cos_series(phi):
    """Exact cos-series of the QSP expectation in float64: g = a0 + sum
    gam_m cos(m theta) via the 2x2 recurrence on a 64-pt grid + rFFT."""
    phi = np.asarray(phi, dtype=np.float64)
    nfft = 64
    theta = 2 * np.pi * np.arange(nfft) / nfft
    x = theta / 2
    c = np.cos(x)
    s = np.sin(x)
    a = np.exp(1j * phi[0]) * np.ones_like(x, dtype=np.complex128)
    b = np.zeros_like(a)
    for k in range(1, NH):
        p = np.exp(1j * phi[k])
        ta = a * c + b * (1j * s)
        tb = a * (1j * s) + b * c
        a = ta * p
        b = tb * np.conj(p)
    g = a.real
    F = np.fft.rfft(g) / nfft
    a0 = F[0].real
    gam = 2 * F.real[1 : DEPTH + 1]
    sin_part = -2 * F.imag[1 : DEPTH + 1]
    assert np.abs(sin_part).max() < 1e-9, "sin components should vanish"
    return float(a0), gam


def _poly_y(a0, gam, K):
    """p(y) coefficients (ascending, float64): p(y) = P(1-2y),
    P(c) = a0 + sum_{m<=K} gam_m T_m(c)."""
    from numpy.polynomial import chebyshev as C, polynomial as Po

    cheb = np.zeros(K + 1)
    cheb[0] = a0
    cheb[1 : K + 1] = gam[:K]
    pc = C.cheb2poly(cheb)
    py = np.zeros(1)
    for i, co in enumerate(pc):
        py = Po.polyadd(py, co * Po.polypow([1.0, -2.0], i))
    return py


def _pick_K(a0, gam):
    rms = np.sqrt(a0**2 + (gam**2).sum() / 2)
    for K in range(4, DEPTH + 1):
        tail = np.sqrt((gam[K:] ** 2).sum() / 2)
        if tail < 6e-3 * rms:
            return K
    return DEPTH


def _factorize(py):
    """p(y) = lead * prod[(y+b)^2 + d] * prod[(y - r)]: complex root pairs
    map directly; real roots pair up in sorted order; an odd leftover
    becomes a linear factor."""
    lead = py[-1]
    roots = np.roots(py[::-1])
    quads = []
    reals = []
    used = np.zeros(len(roots), bool)
    for i, r in enumerate(roots):
        if used[i]:
            continue
        if abs(r.imag) > 1e-9:
            j = np.argmin(np.abs(roots - np.conj(r)) + used * 1e9)
            used[i] = used[j] = True
            quads.append((float(-r.real), float(r.imag**2)))
        else:
            used[i] = True
            reals.append(float(r.real))
    reals.sort()
    while len(reals) >= 2:
        r1 = reals.pop()
        r2 = reals.pop()
        quads.append((float(-(r1 + r2) / 2), float(-((r1 - r2) / 2) ** 2)))
    return float(lead), quads, reals


def _build_nc(quads, lins, tiles=None, tt_pool=None):
    import concourse.bacc as bacc
    import concourse.mybir as mybir
    import concourse.tile as tile

    if tiles is None:
        tiles = TILES
    if tt_pool is None:
        tt_pool = TT_POOL
    ops = _register_custom_ops()

    f16 = mybir.dt.float16
    f32 = mybir.dt.float32
    Sin = mybir.ActivationFunctionType.Sin
    Square = mybir.ActivationFunctionType.Square
    mult = mybir.AluOpType.mult
    add = mybir.AluOpType.add

    fast = len(quads) == 3 and len(lins) == 0

    nc = bacc.Bacc()
    uin = nc.dram_tensor("u", [P, FD], f16, kind="ExternalInput")
    alf = nc.dram_tensor("al", [P, FD], f16, kind="ExternalInput")
    out = nc.dram_tensor("out", [P, FD], f16, kind="ExternalOutput")

    d3_ap = None
    if fast:
        t = nc.alloc_sbuf_tensor("const-d3", [P, 1], f32)
        nc.gpsimd.memset(t.ap(), float(quads[2][1]))
        d3_ap = t.ap()
        # No extra barrier: the memset lands within ~1us on the GpSimd
        # queue while the first PAIR read is >4us out.
    else:
        for b, _ in quads:
            if (f32, float(b)) not in nc.const_aps.aps:
                t = nc.alloc_sbuf_tensor(f"const-b-{b}", [P, 1], f32)
                nc.gpsimd.memset(t.ap(), float(b))
                nc.const_aps.aps[(f32, float(b))] = t.ap()
        nc.all_engine_barrier()

    nt = len(tiles)
    offs = np.concatenate([[0], np.cumsum(tiles)]).astype(int)

    with tile.TileContext(nc) as tc:
        with (
            tc.tile_pool(name="io", bufs=1) as io_pool,
            tc.tile_pool(name="trig", bufs=3) as trig_pool,
            tc.tile_pool(name="sq", bufs=4) as sq_pool,
            tc.tile_pool(name="acc", bufs=4) as acc_pool,
        ):
            # Input DMA order u0,u1,al0,u2,al1,...: u(t+1) lands before al(t)
            # so Sin(t+1) never waits while al(t) still arrives in time for
            # the t-th product. All inputs precede all outputs on SP (DMA
            # instructions hold their queue during sem waits).
            uts, ats = [None] * nt, [None] * nt
            order = [("u", 0)]
            for t in range(1, nt):
                order += [("u", t), ("al", t - 1)]
            order.append(("al", nt - 1))
            for kind, t in order:
                sl = slice(offs[t], offs[t + 1])
                if kind == "u":
                    ut = io_pool.tile([P, tiles[t]], f16, tag=f"ut{t}")
                    nc.sync.dma_start(out=ut[:], in_=uin[:, sl])
                    uts[t] = ut[:]
                else:
                    at = io_pool.tile([P, tiles[t]], f16, tag=f"at{t}")
                    nc.sync.dma_start(out=at[:], in_=alf[:, sl])
                    ats[t] = at[:]
            # All sins up-front so no output DMA issued from the ACT queue
            # can head-of-line block a later Sin dispatch.
            ss = []
            for t in range(nt):
                s = trig_pool.tile([P, tiles[t]], f16, tag=f"s{t % 3}")
                nc.scalar.activation(s[:], uts[t], Sin, bias=0.0, scale=1.0)
                ss.append(s)
            # Output DMA issue rotates across queues so their sem waits (the
            # queue is held while waiting) run in parallel, not as a cascade.
            def out_eng(t):
                return nc.scalar if t == nt - 1 else nc.sync

            for t in range(nt):
                tfd = tiles[t]
                sl = slice(offs[t], offs[t + 1])
                s = ss[t]

                if fast:
                    (b1, d1), (b2, d2), (b3, _) = quads
                    p23 = sq_pool.tile([P, tfd], f16, tag="p23")
                    nc.vector._custom_dve(
                        ops["pair"], out=p23[:], in0=s[:], in1=d3_ap,
                        s0=float(b2), s1=float(d2), imm2=float(b3),
                    )
                    q1 = acc_pool.tile([P, tfd], f16, tag="q1")
                    nc.vector._custom_dve(
                        ops["qm"], out=q1[:], in0=s[:], in1=ats[t],
                        s0=float(b1), s1=float(d1),
                    )
                    ot = acc_pool.tile([P, tfd], f16, tag="ot")
                    eng = nc.gpsimd if t in tt_pool else nc.vector
                    eng.tensor_mul(ot[:], q1[:], p23[:])
                    out_eng(t).dma_start(out=out[:, sl], in_=ot[:])
                else:
                    y = trig_pool.tile([P, tfd], f16, tag="y")
                    nc.vector.tensor_mul(y[:], s[:], s[:])
                    acc_ap = ats[t]
                    for b, d in quads:
                        g = sq_pool.tile([P, tfd], f16, tag="g")
                        nc.scalar.activation(g[:], y[:], Square, bias=float(b),
                                             scale=1.0)
                        nacc = acc_pool.tile([P, tfd], f16, tag="acc")
                        nc.vector.scalar_tensor_tensor(
                            nacc[:], g[:], float(d), acc_ap, add, mult
                        )
                        acc_ap = nacc[:]
                    for r in lins:
                        nacc = acc_pool.tile([P, tfd], f16, tag="acc")
                        nc.vector.scalar_tensor_tensor(
                            nacc[:], y[:], float(-r), acc_ap, add, mult
                        )
                        acc_ap = nacc[:]
                    out_eng(t).dma_start(out=out[:, sl], in_=acc_ap)
    nc.finalize()
    return nc


def _prep(key):
    if key not in _cache:
        phi = np.frombuffer(key, dtype=np.float32)
        a0, gam = _cos_series(phi)
        K = _pick_K(a0, gam)
        py = _poly_y(a0, gam, K)
        lead, quads, lins = _factorize(py)
        nc = _build_nc(quads, lins)
        _cache[key] = (nc, lead)
    return _cache[key]


def _get_runner(key):
    return _prep(key)[0]


def kernel(x, qsp_params, alphas):
    from concourse.bass_utils import run_bass_kernel_spmd

    x = np.asarray(x, dtype=np.float32).reshape(-1)
    alphas = np.asarray(alphas, dtype=np.float32).reshape(-1)
    qsp_params = np.asarray(qsp_params, dtype=np.float32).reshape(-1)
    assert x.shape[0] == N and alphas.shape[0] == N

    nc, lead = _prep(qsp_params.tobytes())

    # Host range reduction (wrap only): u = x - pi*round(x/pi) in [-pi/2,pi/2];
    # cos(2x) = cos(2u) = 1 - 2 sin^2(u). Leading coeff rides on alphas.
    xf = x.astype(np.float64)
    u = (xf - PI64 * np.round(xf / PI64)).astype(np.float16)
    al = (alphas.astype(np.float64) * lead).astype(np.float16)

    pad = P * FD - PER
    in_maps = []
    for c in range(NCORES):
        cs = slice(c * PER, (c + 1) * PER)
        in_maps.append({
            "u": np.pad(u[cs], (0, pad)).reshape(P, FD),
            "al": np.pad(al[cs], (0, pad)).reshape(P, FD),
        })

    res = run_bass_kernel_spmd(nc, in_maps, core_ids=list(range(NCORES)))
    outs = [r["out"].reshape(-1)[:PER] for r in res.results]
    return np.concatenate(outs).astype(np.float32)[:, None]


# revision 28
# speedup vs baseline: 6.6173x; 1.0087x over previous
"""QSP expectation kernel for Trainium2 (Bass/Tile), 8-core data parallel.

Math: Re(U[0,0]) of the QSP sequence is an EVEN trig polynomial of theta=2x
(structural: U00 = P(cos x) has even real part), so with c = cos(2x) the
output is a single degree-K polynomial in c. The host ships only the
range-reduced u = x - pi*round(x/pi) (fp16) and alphas (fp16, pre-scaled by
the leading coefficient); the device computes s = Sin(u) (ScalarE) and
evaluates the polynomial over y = s^2 (c = 1 - 2y) factored into real
quadratics (y+b)^2 + d.

The quadratic factors are fused into two custom DVE ops (registered at
import via the documented dve_ops authoring API; the uop program is written
into the per-NEFF DVE table, no firmware change):

  QSP_QUAD_MUL:  out = ((s^2 + b)^2 + d) * acc        (5 ALU stages)
  QSP_PAIR:      out = ((s^2+b2)^2+d2)*((s^2+b3)^2+d3) (8 stages, d3 via the
                 C3->Src1 latch spill, a [P,1] const AP)

so a degree-6 evaluation is 3 VectorE ops per tile (QUAD_MUL, PAIR, one fp16
tensor_tensor for the final product) plus a single ScalarE Sin. Truncation K
is chosen adaptively (rel L2 truncation err < 6e-3 of signal rms; tolerance
2e-2); non-degree-6 parameter sets fall back to a native Square/STT chain.
"""

import numpy as np

N = 4_000_000
NCORES = 8
PER = N // NCORES          # 500_000 elements per core
P = 128                    # SBUF partitions
FD = 3912                  # free dim per core; PER=500000 padded to P*FD=500736
DEPTH = 10
NH = 2 * DEPTH + 1

# Non-uniform column tiles: small first tile so compute starts early, small
# last tile so the final store+sem tail is short. TT_POOL marks tiles whose
# final product runs on the otherwise-idle GpSimd engine (early tiles, so
# its slower ops drain before the pipeline tail).
TILES = [520, 870, 1141, 1085, 296]
TT_POOL = (0, 1, 2)
assert sum(TILES) == FD

PI64 = np.float64(np.pi)

_cache = {}
_ops = {}


def _register_custom_ops():
    """Register the two fused quadratic-factor ops in concourse.dve_ops via
    the documented authoring pattern (OPS.append + sub-opcode row); shas are
    computed from the actual lowering so compile()'s pin check passes."""
    if _ops:
        return _ops
    import concourse.dve_ops as dve_ops
    from concourse.dve_ops import DveOp
    from concourse.dve_spec import (
        C0, C1, C2, C3, Spec, Src0, Src1, _spill_c3_to_src1, lower, sq,
    )
    from concourse.dve_spec import _has_src1
    from concourse.dve_uop import DveOpSpec

    if "QSP_QUAD_MUL" in dve_ops._SUB_OPCODE_FOR_NAME:
        _ops["qm"] = next(o for o in dve_ops.OPS if o.name == "QSP_QUAD_MUL")
        _ops["pair"] = next(o for o in dve_ops.OPS if o.name == "QSP_PAIR")
        return _ops

    used_rows = set(dve_ops._SUB_OPCODE_FOR_NAME.values())
    try:
        from concourse.dve_table_gen import free_opcode_rows
        free = [r for r in free_opcode_rows("TRN2") if r not in used_rows]
    except Exception:
        free = [r for r in range(1, 0x20) if r not in used_rows]
    rows = free[:2]
    assert len(rows) == 2, "no free DVE opcode rows"

    def ref_qm(in0, in1, s0, s1, imm2):
        y = in0.astype(np.float32) * in0.astype(np.float32)
        return (((y + s0) * (y + s0) + s1) * in1).astype(np.float32)

    def ref_pair(in0, in1, s0, s1, imm2):
        y = in0.astype(np.float32) * in0.astype(np.float32)
        f2 = (y + s0) * (y + s0) + s1
        f3 = (y + imm2) * (y + imm2) + np.asarray(in1, np.float32).reshape(-1, 1)
        return (f2 * f3).astype(np.float32)

    y = Src0 * Src0
    spec_qm = Spec(body=(sq(y + C0) + C1) * Src1, reference=ref_qm)
    spec_pair = Spec(
        body=_spill_c3_to_src1((sq(y + C0) + C1) * (sq(y + C2) + C3)),
        reference=ref_pair,
    )

    made = []
    for name, row, spec in (
        ("QSP_QUAD_MUL", rows[0], spec_qm),
        ("QSP_PAIR", rows[1], spec_pair),
    ):
        dve_ops._SUB_OPCODE_FOR_NAME[name] = row
        shas = {}
        for ver in ("v3", "v4"):
            try:
                d = DveOpSpec(name=name, opcode=row, uops=lower(spec, ver=ver),
                              rd1_en=_has_src1(spec))
                shas[ver] = d.sha(ver)
            except Exception:
                pass
        assert "v3" in shas, f"{name}: v3 lowering failed"
        op = DveOp(name, spec, subdim=False, uops_sha=shas)
        dve_ops.OPS.append(op)
        dve_ops.CUSTOM_DVE_SPECS[name] = spec
        made.append(op)
    _ops["qm"], _ops["pair"] = made
    return _ops


def _cos_series(phi):
    """Exact cos-series of the QSP expectation in float64: g = a0 + sum
    gam_m cos(m theta) via the 2x2 recurrence on a 64-pt grid + rFFT."""
    phi = np.asarray(phi, dtype=np.float64)
    nfft = 64
    theta = 2 * np.pi * np.arange(nfft) / nfft
    x = theta / 2
    c = np.cos(x)
    s = np.sin(x)
    a = np.exp(1j * phi[0]) * np.ones_like(x, dtype=np.complex128)
    b = np.zeros_like(a)
    for k in range(1, NH):
        p = np.exp(1j * phi[k])
        ta = a * c + b * (1j * s)
        tb = a * (1j * s) + b * c
        a = ta * p
        b = tb * np.conj(p)
    g = a.real
    F = np.fft.rfft(g) / nfft
    a0 = F[0].real
    gam = 2 * F.real[1 : DEPTH + 1]
    sin_part = -2 * F.imag[1 : DEPTH + 1]
    assert np.abs(sin_part).max() < 1e-9, "sin components should vanish"
    return float(a0), gam


def _poly_y(a0, gam, K):
    """p(y) coefficients (ascending, float64): p(y) = P(1-2y),
    P(c) = a0 + sum_{m<=K} gam_m T_m(c)."""
    from numpy.polynomial import chebyshev as C, polynomial as Po

    cheb = np.zeros(K + 1)
    cheb[0] = a0
    cheb[1 : K + 1] = gam[:K]
    pc = C.cheb2poly(cheb)
    py = np.zeros(1)
    for i, co in enumerate(pc):
        py = Po.polyadd(py, co * Po.polypow([1.0, -2.0], i))
    return py


def _pick_K(a0, gam):
    rms = np.sqrt(a0**2 + (gam**2).sum() / 2)
    for K in range(4, DEPTH + 1):
        tail = np.sqrt((gam[K:] ** 2).sum() / 2)
        if tail < 6e-3 * rms:
            return K
    return DEPTH


def _factorize(py):
    """p(y) = lead * prod[(y+b)^2 + d] * prod[(y - r)]: complex root pairs
    map directly; real roots pair up in sorted order; an odd leftover
    becomes a linear factor."""
    lead = py[-1]
    roots = np.roots(py[::-1])
    quads = []
    reals = []
    used = np.zeros(len(roots), bool)
    for i, r in enumerate(roots):
        if used[i]:
            continue
        if abs(r.imag) > 1e-9:
            j = np.argmin(np.abs(roots - np.conj(r)) + used * 1e9)
            used[i] = used[j] = True
            quads.append((float(-r.real), float(r.imag**2)))
        else:
            used[i] = True
            reals.append(float(r.real))
    reals.sort()
    while len(reals) >= 2:
        r1 = reals.pop()
        r2 = reals.pop()
        quads.append((float(-(r1 + r2) / 2), float(-((r1 - r2) / 2) ** 2)))
    return float(lead), quads, reals


def _build_nc(quads, lins, tiles=None, tt_pool=None):
    import concourse.bacc as bacc
    import concourse.mybir as mybir
    import concourse.tile as tile

    if tiles is None:
        tiles = TILES
    if tt_pool is None:
        tt_pool = TT_POOL
    ops = _register_custom_ops()

    f16 = mybir.dt.float16
    f32 = mybir.dt.float32
    Sin = mybir.ActivationFunctionType.Sin
    Square = mybir.ActivationFunctionType.Square
    mult = mybir.AluOpType.mult
    add = mybir.AluOpType.add

    fast = len(quads) == 3 and len(lins) == 0

    nc = bacc.Bacc()
    uin = nc.dram_tensor("u", [P, FD], f16, kind="ExternalInput")
    alf = nc.dram_tensor("al", [P, FD], f16, kind="ExternalInput")
    out = nc.dram_tensor("out", [P, FD], f16, kind="ExternalOutput")

    d3_ap = None
    if fast:
        t = nc.alloc_sbuf_tensor("const-d3", [P, 1], f32)
        nc.gpsimd.memset(t.ap(), float(quads[2][1]))
        d3_ap = t.ap()
        # No extra barrier: the memset lands within ~1us on the GpSimd
        # queue while the first PAIR read is >4us out.
    else:
        for b, _ in quads:
            if (f32, float(b)) not in nc.const_aps.aps:
                t = nc.alloc_sbuf_tensor(f"const-b-{b}", [P, 1], f32)
                nc.gpsimd.memset(t.ap(), float(b))
                nc.const_aps.aps[(f32, float(b))] = t.ap()
        nc.all_engine_barrier()

    nt = len(tiles)
    offs = np.concatenate([[0], np.cumsum(tiles)]).astype(int)

    with tile.TileContext(nc) as tc:
        with (
            tc.tile_pool(name="io", bufs=1) as io_pool,
            tc.tile_pool(name="trig", bufs=3) as trig_pool,
            tc.tile_pool(name="sq", bufs=4) as sq_pool,
            tc.tile_pool(name="acc", bufs=4) as acc_pool,
        ):
            # Input DMA order u0,u1,al0,u2,al1,...: u(t+1) lands before al(t)
            # so Sin(t+1) never waits while al(t) still arrives in time for
            # the t-th product. All inputs precede all outputs on SP (DMA
            # instructions hold their queue during sem waits).
            uts, ats = [None] * nt, [None] * nt
            order = [("u", 0)]
            for t in range(1, nt):
                order += [("u", t), ("al", t - 1)]
            order.append(("al", nt - 1))
            for kind, t in order:
                sl = slice(offs[t], offs[t + 1])
                if kind == "u":
                    ut = io_pool.tile([P, tiles[t]], f16, tag=f"ut{t}")
                    nc.sync.dma_start(out=ut[:], in_=uin[:, sl])
                    uts[t] = ut[:]
                else:
                    at = io_pool.tile([P, tiles[t]], f16, tag=f"at{t}")
                    nc.sync.dma_start(out=at[:], in_=alf[:, sl])
                    ats[t] = at[:]
            # All sins up-front so no output DMA issued from the ACT queue
            # can head-of-line block a later Sin dispatch.
            ss = []
            for t in range(nt):
                s = trig_pool.tile([P, tiles[t]], f16, tag=f"s{t % 3}")
                nc.scalar.activation(s[:], uts[t], Sin, bias=0.0, scale=1.0)
                ss.append(s)
            # Output DMA issue rotates across queues so their sem waits (the
            # queue is held while waiting) run in parallel, not as a cascade.
            def out_eng(t):
                return nc.scalar if t == nt - 1 else nc.sync

            for t in range(nt):
                tfd = tiles[t]
                sl = slice(offs[t], offs[t + 1])
                s = ss[t]

                if fast:
                    (b1, d1), (b2, d2), (b3, _) = quads
                    p23 = sq_pool.tile([P, tfd], f16, tag="p23")
                    nc.vector._custom_dve(
                        ops["pair"], out=p23[:], in0=s[:], in1=d3_ap,
                        s0=float(b2), s1=float(d2), imm2=float(b3),
                    )
                    q1 = acc_pool.tile([P, tfd], f16, tag="q1")
                    nc.vector._custom_dve(
                        ops["qm"], out=q1[:], in0=s[:], in1=ats[t],
                        s0=float(b1), s1=float(d1),
                    )
                    ot = acc_pool.tile([P, tfd], f16, tag="ot")
                    eng = nc.gpsimd if t in tt_pool else nc.vector
                    eng.tensor_mul(ot[:], q1[:], p23[:])
                    out_eng(t).dma_start(out=out[:, sl], in_=ot[:])
                else:
                    y = trig_pool.tile([P, tfd], f16, tag="y")
                    nc.vector.tensor_mul(y[:], s[:], s[:])
                    acc_ap = ats[t]
                    for b, d in quads:
                        g = sq_pool.tile([P, tfd], f16, tag="g")
                        nc.scalar.activation(g[:], y[:], Square, bias=float(b),
                                             scale=1.0)
                        nacc = acc_pool.tile([P, tfd], f16, tag="acc")
                        nc.vector.scalar_tensor_tensor(
                            nacc[:], g[:], float(d), acc_ap, add, mult
                        )
                        acc_ap = nacc[:]
                    for r in lins:
                        nacc = acc_pool.tile([P, tfd], f16, tag="acc")
                        nc.vector.scalar_tensor_tensor(
                            nacc[:], y[:], float(-r), acc_ap, add, mult
                        )
                        acc_ap = nacc[:]
                    out_eng(t).dma_start(out=out[:, sl], in_=acc_ap)
    nc.finalize()
    return nc


def _prep(key):
    if key not in _cache:
        phi = np.frombuffer(key, dtype=np.float32)
        a0, gam = _cos_series(phi)
        K = _pick_K(a0, gam)
        py = _poly_y(a0, gam, K)
        lead, quads, lins = _factorize(py)
        nc = _build_nc(quads, lins)
        _cache[key] = (nc, lead)
    return _cache[key]


def _get_runner(key):
    return _prep(key)[0]


def kernel(x, qsp_params, alphas):
    from concourse.bass_utils import run_bass_kernel_spmd

    x = np.asarray(x, dtype=np.float32).reshape(-1)
    alphas = np.asarray(alphas, dtype=np.float32).reshape(-1)
    qsp_params = np.asarray(qsp_params, dtype=np.float32).reshape(-1)
    assert x.shape[0] == N and alphas.shape[0] == N

    nc, lead = _prep(qsp_params.tobytes())

    # Host range reduction (wrap only): u = x - pi*round(x/pi) in [-pi/2,pi/2];
    # cos(2x) = cos(2u) = 1 - 2 sin^2(u). Leading coeff rides on alphas.
    xf = x.astype(np.float64)
    u = (xf - PI64 * np.round(xf / PI64)).astype(np.float16)
    al = (alphas.astype(np.float64) * lead).astype(np.float16)

    pad = P * FD - PER
    in_maps = []
    for c in range(NCORES):
        cs = slice(c * PER, (c + 1) * PER)
        in_maps.append({
            "u": np.pad(u[cs], (0, pad)).reshape(P, FD),
            "al": np.pad(al[cs], (0, pad)).reshape(P, FD),
        })

    res = run_bass_kernel_spmd(nc, in_maps, core_ids=list(range(NCORES)))
    outs = [r["out"].reshape(-1)[:PER] for r in res.results]
    return np.concatenate(outs).astype(np.float32)[:, None]
